# revision 1
# baseline (speedup 1.0000x reference)
"""2D Haar DWT (analysis) kernel for Trainium2, 8 NeuronCores.

Reference computation: per (batch, channel) slice, Y = A @ X @ A.T with A the
512x512 single-level Haar analysis operator (2-tap filters h0=[s,s],
h1=[-s,s], s=1/sqrt(2), stride 2, no wrap for L=2), then the four quadrants
of Y are concatenated along channels: out[b,i,j,:] = [LL|LH|HL|HH].

Because A is 2-tap / stride-2, every output pixel is a +-s^2-weighted sum of
one 2x2 input block:
    hs = x[2i]   + x[2i+1]        (height lowpass,  unscaled)
    hd = x[2i+1] - x[2i]          (height highpass, unscaled)
    LL = k*(hs[2j] + hs[2j+1])    k = s*s
    LH = k*(hd[2j] + hd[2j+1])
    HL = k*(hs[2j+1] - hs[2j])
    HH = k*(hd[2j+1] - hd[2j])
so the kernel is pure elementwise work (memory-bound), no matmul needed.
The host verifies that A has exactly this structure (it is deterministic in
the problem's setup_inputs); if it ever did not, a numpy fallback computes
the general dense transform.

Sharding: data-parallel over batch, 1 image per NeuronCore (8 cores).

Raw bass (no Tile): this container's walrus accepts at most one sync-wait
command per instruction, which the Tile scheduler's emitted sync_info
violates; here every instruction carries at most one sem wait by
construction. Pipeline: SP issues loads (HWDGE), DVE does the 6 adds/subs
per tile, ACT applies the k scale and issues stores (HWDGE), GPSIMD waits
for everything and resets all semaphores so repeated NEFF executions start
from a clean state.
"""

from contextlib import ExitStack

import numpy as np

import concourse.bass as bass
import concourse.mybir as mybir
from concourse import bass_utils
from concourse.instruction_name_ordered_set import InstructionNameOrderedSet


class _Chain:
    """Declare same-engine program-order as nosync dependencies (what Tile
    emits) so the race detector knows consecutive ops on one engine are
    ordered by the engine itself."""

    def __init__(self):
        self.prev = None

    def __call__(self, inst):
        if self.prev is not None:
            inst.ins.set_nosync_dependencies(
                InstructionNameOrderedSet([self.prev])
            )
        self.prev = inst.ins.name
        return inst

_B = 8
_N = 512
_C = 32
_HALF = _N // 2

# tile geometry (per core)
_IB = 2          # i-blocks of 128 output rows each (256 total)
_WCH = 64        # input-width columns per tile
_WB = _N // _WCH # w-chunks
_JCH = _WCH // 2 # output columns per tile

_NB_EO = 4       # input-tile buffers (load lookahead)
_NB_OT = 3       # output-tile buffers

_f32 = mybir.dt.float32
_ADD = mybir.AluOpType.add
_SUB = mybir.AluOpType.subtract


def _build_nc(k: float, repeat: int = 1) -> bass.Bass:
    """Build the per-core kernel. repeat>1 runs the whole DWT that many
    times inside one NEFF (identical output) — used only for timing via
    the wall-clock slope between repeat values."""
    nc = bass.Bass()
    x = nc.dram_tensor("x", [_N, _N, _C], _f32, kind="ExternalInput")
    out = nc.dram_tensor("out", [_HALF, _HALF, 4 * _C], _f32, kind="ExternalOutput")

    # view x rows as (i, even/odd) pairs
    xr = x[:].rearrange("(i e) w c -> i e w c", e=2)  # [256, 2, 512, 32]

    units = [
        (ib, wb) for _ in range(repeat) for ib in range(_IB) for wb in range(_WB)
    ]
    n_units = len(units)

    with ExitStack() as ctx:
        eo = [
            ctx.enter_context(nc.sbuf_tensor(f"eo{i}", [128, 2, _WCH, _C], _f32))
            for i in range(_NB_EO)
        ]
        hs = ctx.enter_context(nc.sbuf_tensor("hs", [128, _WCH, _C], _f32))
        hd = ctx.enter_context(nc.sbuf_tensor("hd", [128, _WCH, _C], _f32))
        ot = [
            ctx.enter_context(nc.sbuf_tensor(f"ot{i}", [128, _JCH, 4, _C], _f32))
            for i in range(_NB_OT)
        ]
        # One load sem per eo slot and one store sem per ot slot: DMA
        # completions across queues are not ordered, so a single cumulative
        # counter could wake a waiter when a *different* load finished. With
        # per-slot lanes (and the slot-free backpressure keeping at most one
        # in-flight DMA per lane) each wait identifies exactly its transfer.
        s_load = [
            ctx.enter_context(nc.semaphore(f"s_load{i}")) for i in range(_NB_EO)
        ]
        s_store = [
            ctx.enter_context(nc.semaphore(f"s_store{i}")) for i in range(_NB_OT)
        ]
        s_eofree = ctx.enter_context(nc.semaphore("s_eofree"))   # DVE done reading eo buf
        s_otready = ctx.enter_context(nc.semaphore("s_otready")) # DVE done writing ot buf
        s_mul = ctx.enter_context(nc.semaphore("s_mul"))         # ACT mul datapath done
        s_bar = ctx.enter_context(nc.semaphore("s_bar"))         # end-of-stream barrier
        block = ctx.enter_context(nc.Block())

        sems = s_load + s_store + [s_eofree, s_otready, s_mul, s_bar]
        n_store_lane = [len(range(lane, n_units, _NB_OT)) for lane in range(_NB_OT)]

        @block.sync
        def _(sync):
            ch = _Chain()
            for u, (ib, wb) in enumerate(units):
                src = xr[
                    ib * 128 : (ib + 1) * 128, :, wb * _WCH : (wb + 1) * _WCH, :
                ]
                i = ch(sync.dma_start(out=eo[u % _NB_EO][:], in_=src))
                if u >= _NB_EO:
                    i.wait_op(s_eofree, u - _NB_EO + 1, "sem-ge")
                i.then_inc(s_load[u % _NB_EO], 16)
            ch(sync.sem_inc(s_bar, 1))

        @block.vector
        def _(vector):
            ch = _Chain()
            for u, (ib, wb) in enumerate(units):
                b = eo[u % _NB_EO]
                o = ot[u % _NB_OT]
                ev = b[:, 0]  # [128, WCH, C]
                od = b[:, 1]
                ch(vector.tensor_tensor(out=hs[:], in0=ev, in1=od, op=_ADD)).wait_op(
                    s_load[u % _NB_EO], 16 * (u // _NB_EO + 1), "sem-ge"
                )
                ch(vector.tensor_tensor(out=hd[:], in0=od, in1=ev, op=_SUB)).then_inc(
                    s_eofree, 1
                )
                sv = hs[:].rearrange("p (j e) c -> p j e c", e=2)
                dv = hd[:].rearrange("p (j e) c -> p j e c", e=2)
                quads = (
                    (sv[:, :, 0], sv[:, :, 1], _ADD),  # LL
                    (dv[:, :, 0], dv[:, :, 1], _ADD),  # LH
                    (sv[:, :, 1], sv[:, :, 0], _SUB),  # HL
                    (dv[:, :, 1], dv[:, :, 0], _SUB),  # HH
                )
                for qi, (a, bb, op) in enumerate(quads):
                    i = ch(vector.tensor_tensor(out=o[:, :, qi], in0=a, in1=bb, op=op))
                    if qi == 0 and u >= _NB_OT:
                        # ot slot reuse: wait until its previous store landed
                        i.wait_op(
                            s_store[u % _NB_OT], 16 * (u // _NB_OT), "sem-ge"
                        )
                i.then_inc(s_otready, 1)
            ch(vector.sem_inc(s_bar, 1))

        @block.scalar
        def _(scalar):
            ch = _Chain()
            for u, (ib, wb) in enumerate(units):
                o = ot[u % _NB_OT]
                otf = o[:].rearrange("p j q c -> p (j q c)")
                i = ch(scalar.mul(otf, otf, k)).wait_op(s_otready, u + 1, "sem-ge")
                i.then_inc(s_mul, 1)
                dst = out[
                    ib * 128 : (ib + 1) * 128, wb * _JCH : (wb + 1) * _JCH, :
                ]
                # the HWDGE trigger would otherwise race the ACT datapath
                ch(scalar.dma_start(
                    out=dst, in_=o[:].rearrange("p j q c -> p j (q c)")
                )).wait_op(s_mul, u + 1, "sem-ge").then_inc(s_store[u % _NB_OT], 16)
            ch(scalar.sem_inc(s_bar, 1))

        @block.gpsimd
        def _(gpsimd):
            ch = _Chain()
            ch(gpsimd.wait_ge(s_bar, 3))
            for lane in range(_NB_OT):
                ch(gpsimd.wait_ge(s_store[lane], 16 * n_store_lane[lane]))
            # observe every semaphore's final value before resetting them
            for lane in range(_NB_EO):
                ch(gpsimd.wait_ge(s_load[lane], 16 * len(range(lane, n_units, _NB_EO))))
            ch(gpsimd.wait_ge(s_eofree, n_units))
            ch(gpsimd.wait_ge(s_otready, n_units))
            ch(gpsimd.wait_ge(s_mul, n_units))
            nums = sorted(s.num for s in sems)
            lo = nums[0]
            hi = nums[-1] + 1
            assert nums == list(range(lo, hi)), nums
            ch(gpsimd.dma_reset(range(lo, hi)))
            ch(gpsimd.sem_clear(range(lo, hi)))

    return nc


def _expected_A(s: np.float32) -> np.ndarray:
    A = np.zeros((_N, _N), np.float32)
    i = np.arange(_HALF)
    A[i, 2 * i] = s
    A[i, 2 * i + 1] = s
    A[_HALF + i, 2 * i] = -s
    A[_HALF + i, 2 * i + 1] = s
    return A


def _fallback(x: np.ndarray, A: np.ndarray) -> np.ndarray:
    # dense separable transform, mirrors the reference in fp32
    xt = np.transpose(x, (0, 2, 1, 3))
    y = np.einsum("ij,bjkc->bikc", A, xt, optimize=True).astype(np.float32)
    y = np.transpose(y, (0, 2, 1, 3))
    y = np.einsum("ij,bjkc->bikc", A, y, optimize=True).astype(np.float32)
    mid = y.shape[1] // 2
    return np.concatenate(
        [y[:, :mid, :mid], y[:, mid:, :mid], y[:, :mid, mid:], y[:, mid:, mid:]],
        axis=-1,
    )


def run_on_device(x: np.ndarray, k: float, trace: bool = False):
    """Run the Bass kernel on 8 cores. Returns (out [8,256,256,128], results)."""
    nc = _build_nc(k)
    in_maps = [{"x": np.ascontiguousarray(x[b])} for b in range(_B)]
    res = bass_utils.run_bass_kernel_spmd(
        nc, in_maps, core_ids=list(range(_B)), trace=trace
    )
    out = np.stack([r["out"] for r in res.results], axis=0)
    return out, res


def kernel(x: np.ndarray, A: np.ndarray) -> np.ndarray:
    x = np.asarray(x, dtype=np.float32)
    A = np.asarray(A, dtype=np.float32)
    s = A[0, 0]
    if not np.array_equal(A, _expected_A(s)):
        return _fallback(x, A)
    k = float(np.float32(s) * np.float32(s))
    out, _ = run_on_device(x, k)
    return out



# revision 2
# speedup vs baseline: 12.2112x; 12.2112x over previous
"""2D Haar DWT (analysis) kernel for Trainium2, 8 NeuronCores.

Reference computation: per (batch, channel) slice, Y = A @ X @ A.T with A the
512x512 single-level Haar analysis operator (2-tap filters h0=[s,s],
h1=[-s,s], s=1/sqrt(2), stride 2, no wrap for L=2), then the four quadrants
of Y are concatenated along channels: out[b,i,j,:] = [LL|LH|HL|HH].

Because A is 2-tap / stride-2, every output pixel is a +-s^2-weighted sum of
one 2x2 input block:
    hs = x[2i]   + x[2i+1]        (height lowpass,  unscaled)
    hd = x[2i+1] - x[2i]          (height highpass, unscaled)
    LL = k*(hs[2j] + hs[2j+1])    k = s*s
    LH = k*(hd[2j] + hd[2j+1])
    HL = k*(hs[2j+1] - hs[2j])
    HH = k*(hd[2j+1] - hd[2j])
so the kernel is pure elementwise work (memory-bound), no matmul needed.

The kernel is HBM-bandwidth bound (measured ~345 GB/s/core with all 8 cores
of a TRN2 device active), so device traffic is halved by running the whole
pipeline in bf16: the host converts x to bf16 before upload and widens the
bf16 result back to fp32 after download. Max Frobenius rel-err from bf16
rounding is ~4e-3, far inside the 2e-2 gate. The host verifies that A has
exactly the expected Haar structure (it is deterministic in the problem's
setup_inputs); if it ever did not, a numpy fallback computes the general
dense transform in fp32.

Sharding: data-parallel over batch, 1 image per NeuronCore (8 cores).

Raw bass (no Tile): this container's walrus accepts at most one sync-wait
command per instruction, which the Tile scheduler's emitted sync_info
violates; here every instruction carries at most one sem wait by
construction. Pipeline: SP issues loads (HWDGE), DVE does the 6 adds/subs
per tile, ACT applies the k scale and issues stores (HWDGE), GPSIMD waits
for everything and resets all semaphores so repeated NEFF executions start
from a clean state.
"""

from contextlib import ExitStack

import numpy as np

import concourse.bass as bass
import concourse.mybir as mybir
from concourse import bass_utils
from concourse.instruction_name_ordered_set import InstructionNameOrderedSet

try:
    import ml_dtypes

    _BF16_NP = np.dtype(ml_dtypes.bfloat16)
except ImportError:  # pragma: no cover
    _BF16_NP = None


class _Chain:
    """Declare same-engine program-order as nosync dependencies (what Tile
    emits) so the race detector knows consecutive ops on one engine are
    ordered by the engine itself."""

    def __init__(self):
        self.prev = None

    def __call__(self, inst):
        if self.prev is not None:
            inst.ins.set_nosync_dependencies(
                InstructionNameOrderedSet([self.prev])
            )
        self.prev = inst.ins.name
        return inst

_B = 8
_N = 512
_C = 32
_HALF = _N // 2

# tile geometry (per core)
_IB = 2          # i-blocks of 128 output rows each (256 total)
_WCH = 64        # input-width columns per tile
_WB = _N // _WCH # w-chunks
_JCH = _WCH // 2 # output columns per tile

_NB_EO = 4       # input-tile buffers (load lookahead)
_NB_OT = 3       # output-tile buffers

_f32 = mybir.dt.float32
_bf16 = mybir.dt.bfloat16
_ADD = mybir.AluOpType.add
_SUB = mybir.AluOpType.subtract


def _build_nc(k: float, repeat: int = 1) -> bass.Bass:
    """Build the per-core kernel (bf16 I/O). repeat>1 runs the whole DWT
    that many times inside one NEFF (identical output) — used only for
    timing via the wall-clock slope between repeat values."""
    nc = bass.Bass()
    x = nc.dram_tensor("x", [_N, _N, _C], _bf16, kind="ExternalInput")
    out = nc.dram_tensor("out", [_HALF, _HALF, 4 * _C], _bf16, kind="ExternalOutput")

    # view x rows as (i, even/odd) pairs
    xr = x[:].rearrange("(i e) w c -> i e w c", e=2)  # [256, 2, 512, 32]

    units = [
        (ib, wb) for _ in range(repeat) for ib in range(_IB) for wb in range(_WB)
    ]
    n_units = len(units)

    with ExitStack() as ctx:
        eo = [
            ctx.enter_context(nc.sbuf_tensor(f"eo{i}", [128, 2, _WCH, _C], _bf16))
            for i in range(_NB_EO)
        ]
        hs = ctx.enter_context(nc.sbuf_tensor("hs", [128, _WCH, _C], _bf16))
        hd = ctx.enter_context(nc.sbuf_tensor("hd", [128, _WCH, _C], _bf16))
        ot = [
            ctx.enter_context(nc.sbuf_tensor(f"ot{i}", [128, _JCH, 4, _C], _bf16))
            for i in range(_NB_OT)
        ]
        # One load sem per eo slot and one store sem per ot slot: DMA
        # completions across queues are not ordered, so a single cumulative
        # counter could wake a waiter when a *different* load finished. With
        # per-slot lanes (and the slot-free backpressure keeping at most one
        # in-flight DMA per lane) each wait identifies exactly its transfer.
        s_load = [
            ctx.enter_context(nc.semaphore(f"s_load{i}")) for i in range(_NB_EO)
        ]
        s_store = [
            ctx.enter_context(nc.semaphore(f"s_store{i}")) for i in range(_NB_OT)
        ]
        s_eofree = ctx.enter_context(nc.semaphore("s_eofree"))   # DVE done reading eo buf
        s_otready = ctx.enter_context(nc.semaphore("s_otready")) # DVE done writing ot buf
        s_mul = ctx.enter_context(nc.semaphore("s_mul"))         # ACT mul datapath done
        s_bar = ctx.enter_context(nc.semaphore("s_bar"))         # end-of-stream barrier
        block = ctx.enter_context(nc.Block())

        sems = s_load + s_store + [s_eofree, s_otready, s_mul, s_bar]
        n_store_lane = [len(range(lane, n_units, _NB_OT)) for lane in range(_NB_OT)]

        @block.sync
        def _(sync):
            ch = _Chain()
            for u, (ib, wb) in enumerate(units):
                src = xr[
                    ib * 128 : (ib + 1) * 128, :, wb * _WCH : (wb + 1) * _WCH, :
                ]
                i = ch(sync.dma_start(out=eo[u % _NB_EO][:], in_=src))
                if u >= _NB_EO:
                    i.wait_op(s_eofree, u - _NB_EO + 1, "sem-ge")
                i.then_inc(s_load[u % _NB_EO], 16)
            ch(sync.sem_inc(s_bar, 1))

        @block.vector
        def _(vector):
            ch = _Chain()
            for u, (ib, wb) in enumerate(units):
                b = eo[u % _NB_EO]
                o = ot[u % _NB_OT]
                ev = b[:, 0]  # [128, WCH, C]
                od = b[:, 1]
                ch(vector.tensor_tensor(out=hs[:], in0=ev, in1=od, op=_ADD)).wait_op(
                    s_load[u % _NB_EO], 16 * (u // _NB_EO + 1), "sem-ge"
                )
                ch(vector.tensor_tensor(out=hd[:], in0=od, in1=ev, op=_SUB)).then_inc(
                    s_eofree, 1
                )
                sv = hs[:].rearrange("p (j e) c -> p j e c", e=2)
                dv = hd[:].rearrange("p (j e) c -> p j e c", e=2)
                quads = (
                    (sv[:, :, 0], sv[:, :, 1], _ADD),  # LL
                    (dv[:, :, 0], dv[:, :, 1], _ADD),  # LH
                    (sv[:, :, 1], sv[:, :, 0], _SUB),  # HL
                    (dv[:, :, 1], dv[:, :, 0], _SUB),  # HH
                )
                for qi, (a, bb, op) in enumerate(quads):
                    i = ch(vector.tensor_tensor(out=o[:, :, qi], in0=a, in1=bb, op=op))
                    if qi == 0 and u >= _NB_OT:
                        # ot slot reuse: wait until its previous store landed
                        i.wait_op(
                            s_store[u % _NB_OT], 16 * (u // _NB_OT), "sem-ge"
                        )
                i.then_inc(s_otready, 1)
            ch(vector.sem_inc(s_bar, 1))

        @block.scalar
        def _(scalar):
            ch = _Chain()
            for u, (ib, wb) in enumerate(units):
                o = ot[u % _NB_OT]
                otf = o[:].rearrange("p j q c -> p (j q c)")
                i = ch(scalar.mul(otf, otf, k)).wait_op(s_otready, u + 1, "sem-ge")
                i.then_inc(s_mul, 1)
                dst = out[
                    ib * 128 : (ib + 1) * 128, wb * _JCH : (wb + 1) * _JCH, :
                ]
                # the HWDGE trigger would otherwise race the ACT datapath
                ch(scalar.dma_start(
                    out=dst, in_=o[:].rearrange("p j q c -> p j (q c)")
                )).wait_op(s_mul, u + 1, "sem-ge").then_inc(s_store[u % _NB_OT], 16)
            ch(scalar.sem_inc(s_bar, 1))

        @block.gpsimd
        def _(gpsimd):
            ch = _Chain()
            ch(gpsimd.wait_ge(s_bar, 3))
            for lane in range(_NB_OT):
                ch(gpsimd.wait_ge(s_store[lane], 16 * n_store_lane[lane]))
            # observe every semaphore's final value before resetting them
            for lane in range(_NB_EO):
                ch(gpsimd.wait_ge(s_load[lane], 16 * len(range(lane, n_units, _NB_EO))))
            ch(gpsimd.wait_ge(s_eofree, n_units))
            ch(gpsimd.wait_ge(s_otready, n_units))
            ch(gpsimd.wait_ge(s_mul, n_units))
            nums = sorted(s.num for s in sems)
            lo = nums[0]
            hi = nums[-1] + 1
            assert nums == list(range(lo, hi)), nums
            ch(gpsimd.dma_reset(range(lo, hi)))
            ch(gpsimd.sem_clear(range(lo, hi)))

    return nc


def _expected_A(s: np.float32) -> np.ndarray:
    A = np.zeros((_N, _N), np.float32)
    i = np.arange(_HALF)
    A[i, 2 * i] = s
    A[i, 2 * i + 1] = s
    A[_HALF + i, 2 * i] = -s
    A[_HALF + i, 2 * i + 1] = s
    return A


def _fallback(x: np.ndarray, A: np.ndarray) -> np.ndarray:
    # dense separable transform, mirrors the reference in fp32
    xt = np.transpose(x, (0, 2, 1, 3))
    y = np.einsum("ij,bjkc->bikc", A, xt, optimize=True).astype(np.float32)
    y = np.transpose(y, (0, 2, 1, 3))
    y = np.einsum("ij,bjkc->bikc", A, y, optimize=True).astype(np.float32)
    mid = y.shape[1] // 2
    return np.concatenate(
        [y[:, :mid, :mid], y[:, mid:, :mid], y[:, :mid, mid:], y[:, mid:, mid:]],
        axis=-1,
    )


def make_in_maps(x: np.ndarray) -> list:
    """Per-core NEFF inputs: bf16-converted batch slices."""
    xb = np.ascontiguousarray(x).astype(_BF16_NP)
    return [{"x": xb[b]} for b in range(_B)]


def run_on_device(x: np.ndarray, k: float, trace: bool = False):
    """Run the Bass kernel on 8 cores. Returns (out [8,256,256,128], results)."""
    nc = _build_nc(k)
    in_maps = make_in_maps(x)
    res = bass_utils.run_bass_kernel_spmd(
        nc, in_maps, core_ids=list(range(_B)), trace=trace
    )
    out = np.stack(
        [np.asarray(r["out"]).astype(np.float32) for r in res.results], axis=0
    )
    return out, res


def kernel(x: np.ndarray, A: np.ndarray) -> np.ndarray:
    x = np.asarray(x, dtype=np.float32)
    A = np.asarray(A, dtype=np.float32)
    s = A[0, 0]
    if _BF16_NP is None or not np.array_equal(A, _expected_A(s)):
        return _fallback(x, A)
    k = float(np.float32(s) * np.float32(s))
    out, _ = run_on_device(x, k)
    return out
